# revision 38
# baseline (speedup 1.0000x reference)
"""Trainium2 Bass kernel for nn_AirspaceModel (126 per-node 2-layer LSTMs).

Sharding: 126 nodes padded to 128, 16 nodes per core across 8 cores
(expert-parallel over the independent per-node LSTMs). Each core keeps its
16 nodes' weights resident in SBUF and runs the full T=24 recurrence for
B=64.

Matmul orientation: gates[b, 4H] = xcatT.T @ W^T — the (transposed)
activations are the stationary operand [K<=128, 64] and the per-node bf16
weights stream as rhs [K, 400]. Nodes are processed in pairs: the even
node's matmul writes PSUM partitions 0-63 and the odd node's partitions
64-127 (tile_position=(0,64)), so the two matmuls run concurrently in
separate array column groups, elementwise ops see full 128-partition
tiles, and the tensor engine runs a pure dense matmul stream (which keeps
the HAM clock gate warm).

Recurrent states are bf16 [128, pair*128] tiles (batch-pair on
partitions; pair block = h(100) + ones col + pad). Each step the new
h1/h2 are transposed back to [*, 128] with ONE batched xbar DMA-transpose
per 4-pair group (zero tensor-engine cost); the layer-0 x rows are DMA'd
over the transpose's pad rows afterwards, and the ones column gives the
bias row that folds biases into the matmuls. Transposed h2 tiles for all
24 steps stay resident in SBUF (52KB) so the linear+conv head at the end
is a short chain of PSUM-accumulating matmuls against host-precomputed
w_lin[h]*w_end[t] columns with no DRAM round trip.

Gate columns are host-reordered to [i, f, o, g] so one sigmoid covers
i,f,o. PSUM accumulation and all elementwise math stay fp32 (cell states
fp32); only matmul operands and h are bf16.
"""

import os
import sys

import numpy as np
import ml_dtypes

for _p in ("/opt/trn_rl_repo", "/root/.axon_site/_ro/trn_rl_repo"):
    if os.path.isdir(_p) and _p not in sys.path:
        sys.path.append(_p)

import concourse.bass as bass
import concourse.mybir as mybir
import concourse.tile as tile
from concourse import bacc
from concourse.bass_utils import run_bass_kernel_spmd
from concourse.masks import make_identity

F32 = mybir.dt.float32
BF16 = mybir.dt.bfloat16
AF = mybir.ActivationFunctionType

N_NODES, SEQ, FEAT, HID, B = 126, 24, 17, 100, 64
NCORES = 8
NP = 128             # padded node count
NN = NP // NCORES    # nodes per core (16)
PAIRS = NN // 2      # node pairs per core (8)
GP = 4               # pairs per pipeline group
NGRP = PAIRS // GP   # groups (2)
H1 = HID + 1         # h rows + ones col/row (101)
K0 = H1 + FEAT       # layer-0 contraction: h1 + ones + x = 118
PB = 128             # state pair-block width (h 0:100, ones 100, pad)

LAST_EXEC_TIME_NS = None
_PROG_CACHE = {}


def _build_program():
    nc = bacc.Bacc("TRN2", target_bir_lowering=False, debug=False)

    xTp = nc.dram_tensor("xTp", [SEQ, FEAT, PAIRS * 128], BF16, kind="ExternalInput")
    w0c = nc.dram_tensor("w0c", [K0, NN * 400], BF16, kind="ExternalInput")
    w1c = nc.dram_tensor("w1c", [H1, NN * 400], BF16, kind="ExternalInput")
    w1h2 = nc.dram_tensor("w1h2", [HID, NN * 400], BF16, kind="ExternalInput")
    wlw = nc.dram_tensor("wlw", [HID, SEQ], BF16, kind="ExternalInput")
    cst = nc.dram_tensor("cst", [1, 1], F32, kind="ExternalInput")
    ones_d = nc.dram_tensor("ones_d", [1, PAIRS * 128], BF16, kind="ExternalInput")
    sel = nc.dram_tensor("sel", [128, 1], F32, kind="ExternalInput")

    outp = nc.dram_tensor("outp", [1, PAIRS * 128], F32, kind="ExternalOutput")
    h1f = nc.dram_tensor("h1f", [128, PAIRS * PB], BF16, kind="ExternalOutput")
    h2f = nc.dram_tensor("h2f", [128, PAIRS * PB], BF16, kind="ExternalOutput")
    c1f = nc.dram_tensor("c1f", [128, PAIRS * HID], F32, kind="ExternalOutput")
    c2f = nc.dram_tensor("c2f", [128, PAIRS * HID], F32, kind="ExternalOutput")

    with tile.TileContext(nc) as tc:
        with (
            tc.tile_pool(name="wpool", bufs=1) as wpool,
            tc.tile_pool(name="state", bufs=1) as state,
            tc.tile_pool(name="xc", bufs=3) as xc,
            tc.tile_pool(name="h2h", bufs=SEQ + 2) as h2h,
            tc.tile_pool(name="ew", bufs=2) as ew,
        ):
            # --- resident weights / constants ----------------------------
            w0c_sb = wpool.tile([K0, NN * 400], BF16)
            w1c_sb = wpool.tile([H1, NN * 400], BF16)
            w1h2_sb = wpool.tile([HID, NN * 400], BF16)
            wlw_sb = wpool.tile([HID, SEQ], BF16)
            cst_sb = wpool.tile([1, 1], F32)
            sel_sb = wpool.tile([128, 1], F32)
            nc.scalar.dma_start(sel_sb[:], sel[:])
            ident = wpool.tile([128, 128], BF16)
            make_identity(nc, ident[:])
            # --- persistent state (batch-pair on partitions, bf16) -------
            # pair p occupies cols p*128..p*128+127: h 0:100, ones col 100
            h1p = state.tile([128, PAIRS * PB], BF16)
            h2p = state.tile([128, PAIRS * PB], BF16)
            c1p = state.tile([128, PAIRS * HID], F32)
            c2p = state.tile([128, PAIRS * HID], F32)
            nc.vector.memset(h1p[:], 0.0)
            nc.vector.memset(
                h1p.rearrange("p (n c) -> p n c", c=PB)[:, :, HID : H1], 1.0)
            nc.vector.memset(h2p[:], 0.0)
            nc.vector.memset(c1p[:], 0.0)
            nc.vector.memset(c2p[:], 0.0)

            # transposed stationary inputs, one [128, 8, 128] tile per step:
            # xh1 rows 0-99 h1T, 100 ones, 101-117 x_t; xh2 rows 0-99 h2T
            xh1_prev = xc.tile([128, PAIRS, 128], BF16, tag="xh1", name="xh1_init")
            nc.vector.memset(xh1_prev[:], 0.0)
            nc.sync.dma_start(
                xh1_prev[HID : H1].rearrange("o p b -> o (p b)"), ones_d[:])
            nc.sync.dma_start(
                xh1_prev[H1:K0].rearrange("o p b -> o (p b)"), xTp[0])

            # weight loads split along partitions (large contiguous rows
            # per descriptor) and spread across the two HWDGE queues; the
            # scalar engine is idle during startup
            def wload(dst, srcT, K, flip):
                for i, lo in enumerate(range(0, K, 32)):
                    hi = min(lo + 32, K)
                    eng = nc.sync if (i + flip) % 2 == 0 else nc.scalar
                    eng.dma_start(dst[lo:hi, :], srcT[lo:hi, :])

            wload(w0c_sb, w0c, K0, 0)
            engs = [nc.scalar, nc.sync, nc.gpsimd]
            q = 0
            for dst, srcT, K in ((w1c_sb, w1c, H1), (w1h2_sb, w1h2, HID)):
                for lo in range(0, K, 32):
                    hi = min(lo + 32, K)
                    engs[q % 3].dma_start(dst[lo:hi, :], srcT[lo:hi, :])
                    q += 1
            nc.scalar.dma_start(wlw_sb[:], wlw[:])
            nc.scalar.dma_start(cst_sb[:], cst[:])

            def ew_acts(gl, sfx):
                """Phase 1: gate activations for a 4-pair group (ACT only).
                gl: PSUM gates [128, 4, 512], gate order i,f,o,g."""
                sf = ew.tile([128, GP * 300], F32, tag="sf" + sfx)
                sf3 = sf.rearrange("p (n c) -> p n c", n=GP)
                tg = ew.tile([128, GP * 100], F32, tag="tg" + sfx)
                tg3 = tg.rearrange("p (n c) -> p n c", n=GP)
                nc.scalar.activation(sf3[:], gl[:, :, 0:300], AF.Sigmoid)
                nc.scalar.activation(tg3[:], gl[:, :, 300:400], AF.Tanh)
                return sf3, tg3

            def ew_state(sf3, tg3, hview, cview, sfx):
                """Phase 2: cell/h update for a 4-pair group."""
                tmp1 = ew.tile([128, GP * 100], F32, tag="t1" + sfx)
                t13 = tmp1.rearrange("p (n c) -> p n c", n=GP)
                tmp2 = ew.tile([128, GP * 100], F32, tag="t2" + sfx)
                t23 = tmp2.rearrange("p (n c) -> p n c", n=GP)
                tcn = ew.tile([128, GP * 100], F32, tag="tc" + sfx)
                tc3 = tcn.rearrange("p (n c) -> p n c", n=GP)
                # c_new = sig(f)*c + sig(i)*tanh(g); all on DVE so the
                # tanh input is ready before ACT reaches it (GPS's ~1us
                # ops otherwise stall the ACT queue via the cell chain)
                nc.vector.tensor_mul(t13[:], sf3[:, :, 100:200], cview)
                nc.vector.tensor_mul(t23[:], sf3[:, :, 0:100], tg3[:])
                nc.vector.tensor_add(cview, t13[:], t23[:])
                nc.scalar.activation(tc3[:], cview, AF.Tanh)
                # h = sig(o)*tanh(c_new), cast to bf16 state
                nc.vector.tensor_mul(hview, sf3[:, :, 200:300], tc3[:])

            h1p3 = h1p.rearrange("p (n c) -> p n c", c=PB)
            h2p3 = h2p.rearrange("p (n c) -> p n c", c=PB)
            c1p3 = c1p.rearrange("p (n c) -> p n c", c=HID)
            c2p3 = c2p.rearrange("p (n c) -> p n c", c=HID)

            with tc.tile_pool(name="gates", bufs=2, space="PSUM") as gpsum:
                h2hist = []
                xh1_hist = {}

                def l0_mms(t, g):
                    gl = gpsum.tile([128, GP, 512], F32, tag="g", name="gl0")
                    for j, p in enumerate(range(g * GP, (g + 1) * GP)):
                        for s in range(2):
                            n = 2 * p + s
                            nc.tensor.matmul(
                                gl[s * 64 : (s + 1) * 64, j, 0:400],
                                xh1_hist[t - 1][0:K0, p, s * 64 : (s + 1) * 64],
                                w0c_sb[:, n * 400 : (n + 1) * 400],
                                start=True, stop=True,
                                tile_position=(0, s * 64),
                            )
                    return gl

                def l1_mms(k, g):
                    gl = gpsum.tile([128, GP, 512], F32, tag="g", name="gl1")
                    for j, p in enumerate(range(g * GP, (g + 1) * GP)):
                        for s in range(2):
                            n = 2 * p + s
                            sl = slice(s * 64, (s + 1) * 64)
                            nc.tensor.matmul(
                                gl[sl, j, 0:400],
                                xh1_hist[k][0:H1, p, sl],
                                w1c_sb[:, n * 400 : (n + 1) * 400],
                                start=True, stop=False,
                                tile_position=(0, s * 64),
                            )
                            nc.tensor.matmul(
                                gl[sl, j, 0:400],
                                h2hist[k][0:HID, p, sl],
                                w1h2_sb[:, n * 400 : (n + 1) * 400],
                                start=False, stop=True,
                                tile_position=(0, s * 64),
                            )
                    return gl

                # half-step staggered pipeline: each (t, g) half-step emits
                # group g's L0(t), L1(t-1), both elementwise phases and the
                # h1/h2 transposes, so the PE always has the other group's
                # matmuls to run while one group's recurrence chain drains
                xh1_hist[-1] = xh1_prev
                for t in range(SEQ):
                    xh1_new = xc.tile([128, PAIRS, 128], BF16, tag="xh1",
                                      name=f"xh1_{t}")
                    xh1_hist[t] = xh1_new
                    if t > 0:
                        xh2_new = h2h.tile([128, PAIRS, 128], BF16,
                                           tag="xh2", name=f"xh2_{t}")
                        h2hist.append(xh2_new)
                    for g in range(NGRP):
                        gsl = slice(g * GP, (g + 1) * GP)
                        csl = slice(g * GP * PB, (g + 1) * GP * PB)
                        gl0 = l0_mms(t, g)
                        if t > 0:
                            gl1 = l1_mms(t - 1, g)
                        elif g == 0:
                            xh2_0 = h2h.tile([128, PAIRS, 128], BF16,
                                             tag="xh2", name="xh2_0")
                            nc.vector.memset(xh2_0[:], 0.0)
                            h2hist.append(xh2_0)
                        a0 = ew_acts(gl0, "a")
                        if t > 0:
                            a1 = ew_acts(gl1, "b")
                        ew_state(*a0, h1p3[:, gsl, 0:HID], c1p3[:, gsl, :],
                                 "a")
                        tpa = gpsum.tile([128, GP, 512], BF16, tag="g",
                                         name="tpa")
                        for j, p in enumerate(range(g * GP, (g + 1) * GP)):
                            nc.tensor.transpose(
                                tpa[0:128, j, 0:128],
                                h1p[:, p * PB : (p + 1) * PB],
                                ident[:],
                            )
                        nc.vector.tensor_copy(
                            xh1_new[0:H1, gsl, :], tpa[0:H1, 0:GP, 0:128])
                        if t + 1 < SEQ:
                            nc.sync.dma_start(
                                xh1_new[H1:K0, gsl, :].rearrange(
                                    "o p b -> o (p b)"),
                                xTp[t + 1][:, g * GP * 128 : (g + 1) * GP * 128],
                            )
                        if t > 0:
                            ew_state(*a1, h2p3[:, gsl, 0:HID],
                                     c2p3[:, gsl, :], "b")
                            nc.sync.dma_start_transpose(
                                xh2_new[0:128, gsl, 0:128], h2p[:, csl])

                # epilogue: layer 1 of the final step
                xh2_new = h2h.tile([128, PAIRS, 128], BF16, tag="xh2",
                                   name=f"xh2_{SEQ}")
                h2hist.append(xh2_new)
                for g in range(NGRP):
                    gsl = slice(g * GP, (g + 1) * GP)
                    csl = slice(g * GP * PB, (g + 1) * GP * PB)
                    gl1 = l1_mms(SEQ - 1, g)
                    a1 = ew_acts(gl1, "b")
                    ew_state(*a1, h2p3[:, gsl, 0:HID], c2p3[:, gsl, :], "b")
                    nc.sync.dma_start_transpose(
                        xh2_new[0:128, gsl, 0:128], h2p[:, csl])

            # --- head: out = sum_t wlw[:,t] . h2T(t) + cst ---------------
            # h2hist[t+1] holds h2T(t). The M=1 matmuls are packed 4-way
            # into PE column groups (psum rows 0/32/64/96, one per t mod 4)
            # so they run concurrently; a final exact-fp32 matmul against a
            # 4-hot selection vector sums the four partial rows.
            with tc.tile_pool(name="hps", bufs=1, space="PSUM") as hps:
                hp = hps.tile([128, PAIRS * 128], F32)
                for t in range(SEQ):
                    r = 32 * (t % 4)
                    h2c = h2hist[t + 1][0:HID].rearrange("o p b -> o (p b)")
                    for half in range(2):
                        nc.tensor.matmul(
                            hp[r : r + 1, half * 512 : (half + 1) * 512],
                            wlw_sb[:, t : t + 1],
                            h2c[:, half * 512 : (half + 1) * 512],
                            start=(t < 4),
                            stop=(t >= SEQ - 4),
                            tile_position=(0, r),
                        )
                hsb = ew.tile([128, PAIRS * 128], F32, tag="hsb")
                nc.vector.tensor_copy(hsb[:], hp[:])
                hp2 = hps.tile([128, PAIRS * 128], F32, name="hp2")
                for half in range(2):
                    nc.tensor.matmul(
                        hp2[0:1, half * 512 : (half + 1) * 512],
                        sel_sb[:],
                        hsb[:, half * 512 : (half + 1) * 512],
                        start=True, stop=True,
                    )
                out_sb = ew.tile([1, PAIRS * 128], F32, tag="out_sb")
                nc.scalar.activation(
                    out_sb[:], hp2[0:1, :], AF.Identity, bias=cst_sb[0:1, 0:1])
                nc.sync.dma_start(outp[:], out_sb[:])

            # --- final states --------------------------------------------
            nc.sync.dma_start(h1f[:], h1p[:])
            nc.sync.dma_start(h2f[:], h2p[:])
            nc.sync.dma_start(c1f[:], c1p[:])
            nc.sync.dma_start(c2f[:], c2p[:])

    nc.compile()
    return nc


def _host_prep(x, W_ih0, W_hh0, b_ih0, b_hh0, W_ih1, W_hh1, b_ih1, b_hh1,
               w_lin, b_lin, w_end, b_end):
    """Pad to 128 nodes and build per-core input maps."""
    def pad_nodes(a):
        pad = [(0, 0)] * a.ndim
        pad[0] = (0, NP - N_NODES)
        return np.pad(a, pad)

    xp = np.pad(x, [(0, 0), (0, 0), (0, NP - N_NODES), (0, 0)])  # [B,T,NP,F]

    # reorder gate blocks [i, f, g, o] -> [i, f, o, g] so one device-side
    # sigmoid covers i,f,o contiguously
    gperm = np.r_[0:200, 300:400, 200:300]
    Wih0, Whh0 = pad_nodes(W_ih0), pad_nodes(W_hh0)
    Wih1, Whh1 = pad_nodes(W_ih1), pad_nodes(W_hh1)
    b0 = pad_nodes(b_ih0 + b_hh0)
    b1 = pad_nodes(b_ih1 + b_hh1)

    wlw = np.outer(w_lin[0], w_end[0]).astype(np.float32)  # [H, T]
    cst = np.array([[b_lin[0] * w_end[0].sum() + b_end[0]]], dtype=np.float32)
    ones = np.ones((1, PAIRS * 128), dtype=np.float32)
    sel = np.zeros((128, 1), dtype=np.float32)
    sel[[0, 32, 64, 96], 0] = 1.0

    in_maps = []
    for c in range(NCORES):
        sl = slice(c * NN, (c + 1) * NN)
        # xTp[t, f, p*128 + s*64 + b] = x[b, t, node, f], node = 16c+2p+s
        xTp = (xp[:, :, sl, :]                 # [B, T, 16, F]
               .transpose(1, 3, 2, 0)          # [T, F, 16, B]
               .reshape(SEQ, FEAT, PAIRS * 128))
        w0 = np.concatenate(
            [Whh0[sl].transpose(2, 0, 1), b0[sl][None],
             Wih0[sl].transpose(2, 0, 1)], axis=0)[:, :, gperm]
        w0 = w0.reshape(K0, NN * 400)
        w1 = np.concatenate(
            [Wih1[sl].transpose(2, 0, 1), b1[sl][None]], axis=0
        )[:, :, gperm].reshape(H1, NN * 400)
        wh2 = Whh1[sl].transpose(2, 0, 1)[:, :, gperm].reshape(HID, NN * 400)
        bf = ml_dtypes.bfloat16
        in_maps.append({
            "xTp": np.ascontiguousarray(xTp).astype(bf),
            "w0c": np.ascontiguousarray(w0).astype(bf),
            "w1c": np.ascontiguousarray(w1).astype(bf),
            "w1h2": np.ascontiguousarray(wh2).astype(bf),
            "wlw": wlw.astype(bf),
            "cst": cst,
            "ones_d": ones.astype(bf),
            "sel": sel,
        })
    return in_maps


def kernel(x, W_ih0, W_hh0, b_ih0, b_hh0, W_ih1, W_hh1, b_ih1, b_hh1,
           w_lin, b_lin, w_end, b_end):
    global LAST_EXEC_TIME_NS
    args = (x, W_ih0, W_hh0, b_ih0, b_hh0, W_ih1, W_hh1, b_ih1, b_hh1,
            w_lin, b_lin, w_end, b_end)
    args = tuple(np.asarray(a, dtype=np.float32) for a in args)
    in_maps = _host_prep(*args)

    if "prog" not in _PROG_CACHE:
        _PROG_CACHE["prog"] = _build_program()
    nc = _PROG_CACHE["prog"]

    trace = os.environ.get("KERNEL_TRACE", "0") == "1"
    try:
        res = run_bass_kernel_spmd(
            nc, in_maps, core_ids=list(range(NCORES)), trace=trace
        )
    except Exception:
        # transient NRT_EXEC_UNIT_UNRECOVERABLE device wedges clear on retry
        res = run_bass_kernel_spmd(
            nc, in_maps, core_ids=list(range(NCORES)), trace=trace
        )
    LAST_EXEC_TIME_NS = res.exec_time_ns

    out = np.zeros((B, 1, N_NODES, 1), dtype=np.float32)
    hn = np.zeros((N_NODES, 2, B, HID), dtype=np.float32)
    cn = np.zeros((N_NODES, 2, B, HID), dtype=np.float32)
    for c in range(NCORES):
        r = res.results[c]
        nlo, nhi = c * NN, min((c + 1) * NN, N_NODES)
        cnt = nhi - nlo

        # outp: [1, p*128 + s*64 + b] -> [node, b]
        o = r["outp"].reshape(PAIRS, 2, B).reshape(NN, B)
        out[:, 0, nlo:nhi, 0] = o[:cnt].T

        def hstates(a):  # bf16 [128, P*128] -> [node, b, h]
            v = np.asarray(a, dtype=np.float32).reshape(2, B, PAIRS, PB)
            return v[..., 0:HID].transpose(2, 0, 1, 3).reshape(NN, B, HID)

        def cstates(a):  # f32 [128, P*100] -> [node, b, h]
            v = a.reshape(2, B, PAIRS, HID)
            return v.transpose(2, 0, 1, 3).reshape(NN, B, HID)

        hn[nlo:nhi, 0] = hstates(r["h1f"])[:cnt]
        hn[nlo:nhi, 1] = hstates(r["h2f"])[:cnt]
        cn[nlo:nhi, 0] = cstates(r["c1f"])[:cnt]
        cn[nlo:nhi, 1] = cstates(r["c2f"])[:cnt]
    return (out, hn, cn)


# revision 39
# speedup vs baseline: 1.0030x; 1.0030x over previous
"""Trainium2 Bass kernel for nn_AirspaceModel (126 per-node 2-layer LSTMs).

Sharding: 126 nodes padded to 128, 16 nodes per core across 8 cores
(expert-parallel over the independent per-node LSTMs). Each core keeps its
16 nodes' weights resident in SBUF and runs the full T=24 recurrence for
B=64.

Matmul orientation: gates[b, 4H] = xcatT.T @ W^T — the (transposed)
activations are the stationary operand [K<=128, 64] and the per-node bf16
weights stream as rhs [K, 400]. Nodes are processed in pairs: the even
node's matmul writes PSUM partitions 0-63 and the odd node's partitions
64-127 (tile_position=(0,64)), so the two matmuls run concurrently in
separate array column groups, elementwise ops see full 128-partition
tiles, and the tensor engine runs a pure dense matmul stream (which keeps
the HAM clock gate warm).

Recurrent states are bf16 [128, pair*128] tiles (batch-pair on
partitions; pair block = h(100) + ones col + pad). Each step the new
h1/h2 are transposed back to [*, 128] with ONE batched xbar DMA-transpose
per 4-pair group (zero tensor-engine cost); the layer-0 x rows are DMA'd
over the transpose's pad rows afterwards, and the ones column gives the
bias row that folds biases into the matmuls. Transposed h2 tiles for all
24 steps stay resident in SBUF (52KB) so the linear+conv head at the end
is a short chain of PSUM-accumulating matmuls against host-precomputed
w_lin[h]*w_end[t] columns with no DRAM round trip.

Gate columns are host-reordered to [i, f, o, g] so one sigmoid covers
i,f,o. PSUM accumulation and all elementwise math stay fp32 (cell states
fp32); only matmul operands and h are bf16.
"""

import os
import sys

import numpy as np
import ml_dtypes

for _p in ("/opt/trn_rl_repo", "/root/.axon_site/_ro/trn_rl_repo"):
    if os.path.isdir(_p) and _p not in sys.path:
        sys.path.append(_p)

import concourse.bass as bass
import concourse.mybir as mybir
import concourse.tile as tile
from concourse import bacc
from concourse.bass_utils import run_bass_kernel_spmd
from concourse.masks import make_identity

F32 = mybir.dt.float32
BF16 = mybir.dt.bfloat16
AF = mybir.ActivationFunctionType

N_NODES, SEQ, FEAT, HID, B = 126, 24, 17, 100, 64
NCORES = 8
NP = 128             # padded node count
NN = NP // NCORES    # nodes per core (16)
PAIRS = NN // 2      # node pairs per core (8)
GP = 4               # pairs per pipeline group
NGRP = PAIRS // GP   # groups (2)
H1 = HID + 1         # h rows + ones col/row (101)
K0 = H1 + FEAT       # layer-0 contraction: h1 + ones + x = 118
PB = 128             # state pair-block width (h 0:100, ones 100, pad)

LAST_EXEC_TIME_NS = None
_PROG_CACHE = {}


def _build_program():
    nc = bacc.Bacc("TRN2", target_bir_lowering=False, debug=False)

    xTp = nc.dram_tensor("xTp", [SEQ, FEAT, PAIRS * 128], BF16, kind="ExternalInput")
    w0c = nc.dram_tensor("w0c", [K0, NN * 400], BF16, kind="ExternalInput")
    w1c = nc.dram_tensor("w1c", [H1, NN * 400], BF16, kind="ExternalInput")
    w1h2 = nc.dram_tensor("w1h2", [HID, NN * 400], BF16, kind="ExternalInput")
    wlw = nc.dram_tensor("wlw", [HID, SEQ], BF16, kind="ExternalInput")
    cst = nc.dram_tensor("cst", [1, 1], F32, kind="ExternalInput")
    ones_d = nc.dram_tensor("ones_d", [1, PAIRS * 128], BF16, kind="ExternalInput")

    outp = nc.dram_tensor("outp", [1, PAIRS * 128], F32, kind="ExternalOutput")
    h1f = nc.dram_tensor("h1f", [128, PAIRS * PB], BF16, kind="ExternalOutput")
    h2f = nc.dram_tensor("h2f", [128, PAIRS * PB], BF16, kind="ExternalOutput")
    c1f = nc.dram_tensor("c1f", [128, PAIRS * HID], F32, kind="ExternalOutput")
    c2f = nc.dram_tensor("c2f", [128, PAIRS * HID], F32, kind="ExternalOutput")

    with tile.TileContext(nc) as tc:
        with (
            tc.tile_pool(name="wpool", bufs=1) as wpool,
            tc.tile_pool(name="state", bufs=1) as state,
            tc.tile_pool(name="xc", bufs=3) as xc,
            tc.tile_pool(name="h2h", bufs=SEQ + 2) as h2h,
            tc.tile_pool(name="ew", bufs=2) as ew,
        ):
            # --- resident weights / constants ----------------------------
            w0c_sb = wpool.tile([K0, NN * 400], BF16)
            w1c_sb = wpool.tile([H1, NN * 400], BF16)
            w1h2_sb = wpool.tile([HID, NN * 400], BF16)
            wlw_sb = wpool.tile([HID, SEQ], BF16)
            cst_sb = wpool.tile([1, 1], F32)
            ident = wpool.tile([128, 128], BF16)
            make_identity(nc, ident[:])
            # --- persistent state (batch-pair on partitions, bf16) -------
            # pair p occupies cols p*128..p*128+127: h 0:100, ones col 100
            h1p = state.tile([128, PAIRS * PB], BF16)
            h2p = state.tile([128, PAIRS * PB], BF16)
            c1p = state.tile([128, PAIRS * HID], F32)
            c2p = state.tile([128, PAIRS * HID], F32)
            nc.vector.memset(h1p[:], 0.0)
            nc.vector.memset(
                h1p.rearrange("p (n c) -> p n c", c=PB)[:, :, HID : H1], 1.0)
            nc.vector.memset(h2p[:], 0.0)
            nc.vector.memset(c1p[:], 0.0)
            nc.vector.memset(c2p[:], 0.0)

            # transposed stationary inputs, one [128, 8, 128] tile per step:
            # xh1 rows 0-99 h1T, 100 ones, 101-117 x_t; xh2 rows 0-99 h2T
            xh1_prev = xc.tile([128, PAIRS, 128], BF16, tag="xh1", name="xh1_init")
            nc.vector.memset(xh1_prev[:], 0.0)
            nc.sync.dma_start(
                xh1_prev[HID : H1].rearrange("o p b -> o (p b)"), ones_d[:])
            nc.sync.dma_start(
                xh1_prev[H1:K0].rearrange("o p b -> o (p b)"), xTp[0])

            # weight loads split along partitions (large contiguous rows
            # per descriptor) and spread across the two HWDGE queues; the
            # scalar engine is idle during startup
            def wload(dst, srcT, K, flip):
                for i, lo in enumerate(range(0, K, 32)):
                    hi = min(lo + 32, K)
                    eng = nc.sync if (i + flip) % 2 == 0 else nc.scalar
                    eng.dma_start(dst[lo:hi, :], srcT[lo:hi, :])

            wload(w0c_sb, w0c, K0, 0)
            engs = [nc.scalar, nc.sync, nc.gpsimd]
            q = 0
            for dst, srcT, K in ((w1c_sb, w1c, H1), (w1h2_sb, w1h2, HID)):
                for lo in range(0, K, 32):
                    hi = min(lo + 32, K)
                    engs[q % 3].dma_start(dst[lo:hi, :], srcT[lo:hi, :])
                    q += 1
            nc.scalar.dma_start(wlw_sb[:], wlw[:])
            nc.scalar.dma_start(cst_sb[:], cst[:])

            def ew_acts(gl, sfx):
                """Phase 1: gate activations for a 4-pair group (ACT only).
                gl: PSUM gates [128, 4, 512], gate order i,f,o,g."""
                sf = ew.tile([128, GP * 300], F32, tag="sf" + sfx)
                sf3 = sf.rearrange("p (n c) -> p n c", n=GP)
                tg = ew.tile([128, GP * 100], F32, tag="tg" + sfx)
                tg3 = tg.rearrange("p (n c) -> p n c", n=GP)
                nc.scalar.activation(sf3[:], gl[:, :, 0:300], AF.Sigmoid)
                nc.scalar.activation(tg3[:], gl[:, :, 300:400], AF.Tanh)
                return sf3, tg3

            def ew_state(sf3, tg3, hview, cview, sfx):
                """Phase 2: cell/h update for a 4-pair group."""
                tmp1 = ew.tile([128, GP * 100], F32, tag="t1" + sfx)
                t13 = tmp1.rearrange("p (n c) -> p n c", n=GP)
                tmp2 = ew.tile([128, GP * 100], F32, tag="t2" + sfx)
                t23 = tmp2.rearrange("p (n c) -> p n c", n=GP)
                tcn = ew.tile([128, GP * 100], F32, tag="tc" + sfx)
                tc3 = tcn.rearrange("p (n c) -> p n c", n=GP)
                # c_new = sig(f)*c + sig(i)*tanh(g); all on DVE so the
                # tanh input is ready before ACT reaches it (GPS's ~1us
                # ops otherwise stall the ACT queue via the cell chain)
                nc.vector.tensor_mul(t13[:], sf3[:, :, 100:200], cview)
                nc.vector.tensor_mul(t23[:], sf3[:, :, 0:100], tg3[:])
                nc.vector.tensor_add(cview, t13[:], t23[:])
                nc.scalar.activation(tc3[:], cview, AF.Tanh)
                # h = sig(o)*tanh(c_new), cast to bf16 state
                nc.vector.tensor_mul(hview, sf3[:, :, 200:300], tc3[:])

            h1p3 = h1p.rearrange("p (n c) -> p n c", c=PB)
            h2p3 = h2p.rearrange("p (n c) -> p n c", c=PB)
            c1p3 = c1p.rearrange("p (n c) -> p n c", c=HID)
            c2p3 = c2p.rearrange("p (n c) -> p n c", c=HID)

            with tc.tile_pool(name="gates", bufs=2, space="PSUM") as gpsum:
                h2hist = []
                xh1_hist = {}

                def l0_mms(t, g):
                    gl = gpsum.tile([128, GP, 512], F32, tag="g", name="gl0")
                    for j, p in enumerate(range(g * GP, (g + 1) * GP)):
                        for s in range(2):
                            n = 2 * p + s
                            nc.tensor.matmul(
                                gl[s * 64 : (s + 1) * 64, j, 0:400],
                                xh1_hist[t - 1][0:K0, p, s * 64 : (s + 1) * 64],
                                w0c_sb[:, n * 400 : (n + 1) * 400],
                                start=True, stop=True,
                                tile_position=(0, s * 64),
                            )
                    return gl

                def l1_mms(k, g):
                    gl = gpsum.tile([128, GP, 512], F32, tag="g", name="gl1")
                    for j, p in enumerate(range(g * GP, (g + 1) * GP)):
                        for s in range(2):
                            n = 2 * p + s
                            sl = slice(s * 64, (s + 1) * 64)
                            nc.tensor.matmul(
                                gl[sl, j, 0:400],
                                xh1_hist[k][0:H1, p, sl],
                                w1c_sb[:, n * 400 : (n + 1) * 400],
                                start=True, stop=False,
                                tile_position=(0, s * 64),
                            )
                            nc.tensor.matmul(
                                gl[sl, j, 0:400],
                                h2hist[k][0:HID, p, sl],
                                w1h2_sb[:, n * 400 : (n + 1) * 400],
                                start=False, stop=True,
                                tile_position=(0, s * 64),
                            )
                    return gl

                # half-step staggered pipeline: each (t, g) half-step emits
                # group g's L0(t), L1(t-1), both elementwise phases and the
                # h1/h2 transposes, so the PE always has the other group's
                # matmuls to run while one group's recurrence chain drains
                xh1_hist[-1] = xh1_prev
                for t in range(SEQ):
                    xh1_new = xc.tile([128, PAIRS, 128], BF16, tag="xh1",
                                      name=f"xh1_{t}")
                    xh1_hist[t] = xh1_new
                    if t > 0:
                        xh2_new = h2h.tile([128, PAIRS, 128], BF16,
                                           tag="xh2", name=f"xh2_{t}")
                        h2hist.append(xh2_new)
                    for g in range(NGRP):
                        gsl = slice(g * GP, (g + 1) * GP)
                        csl = slice(g * GP * PB, (g + 1) * GP * PB)
                        gl0 = l0_mms(t, g)
                        if t > 0:
                            gl1 = l1_mms(t - 1, g)
                        elif g == 0:
                            xh2_0 = h2h.tile([128, PAIRS, 128], BF16,
                                             tag="xh2", name="xh2_0")
                            nc.vector.memset(xh2_0[:], 0.0)
                            h2hist.append(xh2_0)
                        a0 = ew_acts(gl0, "a")
                        if t > 0:
                            a1 = ew_acts(gl1, "b")
                        ew_state(*a0, h1p3[:, gsl, 0:HID], c1p3[:, gsl, :],
                                 "a")
                        tpa = gpsum.tile([128, GP, 512], BF16, tag="g",
                                         name="tpa")
                        for j, p in enumerate(range(g * GP, (g + 1) * GP)):
                            nc.tensor.transpose(
                                tpa[0:128, j, 0:128],
                                h1p[:, p * PB : (p + 1) * PB],
                                ident[:],
                            )
                        nc.vector.tensor_copy(
                            xh1_new[0:H1, gsl, :], tpa[0:H1, 0:GP, 0:128])
                        if t + 1 < SEQ:
                            nc.sync.dma_start(
                                xh1_new[H1:K0, gsl, :].rearrange(
                                    "o p b -> o (p b)"),
                                xTp[t + 1][:, g * GP * 128 : (g + 1) * GP * 128],
                            )
                        if t > 0:
                            ew_state(*a1, h2p3[:, gsl, 0:HID],
                                     c2p3[:, gsl, :], "b")
                            nc.sync.dma_start_transpose(
                                xh2_new[0:128, gsl, 0:128], h2p[:, csl])

                # epilogue: layer 1 of the final step
                xh2_new = h2h.tile([128, PAIRS, 128], BF16, tag="xh2",
                                   name=f"xh2_{SEQ}")
                h2hist.append(xh2_new)
                for g in range(NGRP):
                    gsl = slice(g * GP, (g + 1) * GP)
                    csl = slice(g * GP * PB, (g + 1) * GP * PB)
                    gl1 = l1_mms(SEQ - 1, g)
                    a1 = ew_acts(gl1, "b")
                    ew_state(*a1, h2p3[:, gsl, 0:HID], c2p3[:, gsl, :], "b")
                    nc.sync.dma_start_transpose(
                        xh2_new[0:128, gsl, 0:128], h2p[:, csl])

            # --- head: out = sum_t wlw[:,t] . h2T(t) + cst ---------------
            # h2hist[t+1] holds h2T(t) (h2hist[0] is the zero init tile)
            with tc.tile_pool(name="hps", bufs=1, space="PSUM") as hps:
                hp = hps.tile([1, PAIRS * 128], F32)
                for t in range(SEQ):
                    h2c = h2hist[t + 1][0:HID].rearrange("o p b -> o (p b)")
                    for half in range(2):
                        nc.tensor.matmul(
                            hp[:, half * 512 : (half + 1) * 512],
                            wlw_sb[:, t : t + 1],
                            h2c[:, half * 512 : (half + 1) * 512],
                            start=(t == 0),
                            stop=(t == SEQ - 1),
                        )
                out_sb = ew.tile([1, PAIRS * 128], F32, tag="out_sb")
                nc.scalar.activation(
                    out_sb[:], hp[:], AF.Identity, bias=cst_sb[0:1, 0:1])
                nc.sync.dma_start(outp[:], out_sb[:])

            # --- final states --------------------------------------------
            nc.sync.dma_start(h1f[:], h1p[:])
            nc.sync.dma_start(h2f[:], h2p[:])
            nc.sync.dma_start(c1f[:], c1p[:])
            nc.sync.dma_start(c2f[:], c2p[:])

    nc.compile()
    return nc


def _host_prep(x, W_ih0, W_hh0, b_ih0, b_hh0, W_ih1, W_hh1, b_ih1, b_hh1,
               w_lin, b_lin, w_end, b_end):
    """Pad to 128 nodes and build per-core input maps."""
    def pad_nodes(a):
        pad = [(0, 0)] * a.ndim
        pad[0] = (0, NP - N_NODES)
        return np.pad(a, pad)

    xp = np.pad(x, [(0, 0), (0, 0), (0, NP - N_NODES), (0, 0)])  # [B,T,NP,F]

    # reorder gate blocks [i, f, g, o] -> [i, f, o, g] so one device-side
    # sigmoid covers i,f,o contiguously
    gperm = np.r_[0:200, 300:400, 200:300]
    Wih0, Whh0 = pad_nodes(W_ih0), pad_nodes(W_hh0)
    Wih1, Whh1 = pad_nodes(W_ih1), pad_nodes(W_hh1)
    b0 = pad_nodes(b_ih0 + b_hh0)
    b1 = pad_nodes(b_ih1 + b_hh1)

    wlw = np.outer(w_lin[0], w_end[0]).astype(np.float32)  # [H, T]
    cst = np.array([[b_lin[0] * w_end[0].sum() + b_end[0]]], dtype=np.float32)
    ones = np.ones((1, PAIRS * 128), dtype=np.float32)

    in_maps = []
    for c in range(NCORES):
        sl = slice(c * NN, (c + 1) * NN)
        # xTp[t, f, p*128 + s*64 + b] = x[b, t, node, f], node = 16c+2p+s
        xTp = (xp[:, :, sl, :]                 # [B, T, 16, F]
               .transpose(1, 3, 2, 0)          # [T, F, 16, B]
               .reshape(SEQ, FEAT, PAIRS * 128))
        w0 = np.concatenate(
            [Whh0[sl].transpose(2, 0, 1), b0[sl][None],
             Wih0[sl].transpose(2, 0, 1)], axis=0)[:, :, gperm]
        w0 = w0.reshape(K0, NN * 400)
        w1 = np.concatenate(
            [Wih1[sl].transpose(2, 0, 1), b1[sl][None]], axis=0
        )[:, :, gperm].reshape(H1, NN * 400)
        wh2 = Whh1[sl].transpose(2, 0, 1)[:, :, gperm].reshape(HID, NN * 400)
        bf = ml_dtypes.bfloat16
        in_maps.append({
            "xTp": np.ascontiguousarray(xTp).astype(bf),
            "w0c": np.ascontiguousarray(w0).astype(bf),
            "w1c": np.ascontiguousarray(w1).astype(bf),
            "w1h2": np.ascontiguousarray(wh2).astype(bf),
            "wlw": wlw.astype(bf),
            "cst": cst,
            "ones_d": ones.astype(bf),
        })
    return in_maps


def kernel(x, W_ih0, W_hh0, b_ih0, b_hh0, W_ih1, W_hh1, b_ih1, b_hh1,
           w_lin, b_lin, w_end, b_end):
    global LAST_EXEC_TIME_NS
    args = (x, W_ih0, W_hh0, b_ih0, b_hh0, W_ih1, W_hh1, b_ih1, b_hh1,
            w_lin, b_lin, w_end, b_end)
    args = tuple(np.asarray(a, dtype=np.float32) for a in args)
    in_maps = _host_prep(*args)

    if "prog" not in _PROG_CACHE:
        _PROG_CACHE["prog"] = _build_program()
    nc = _PROG_CACHE["prog"]

    trace = os.environ.get("KERNEL_TRACE", "0") == "1"
    try:
        res = run_bass_kernel_spmd(
            nc, in_maps, core_ids=list(range(NCORES)), trace=trace
        )
    except Exception:
        # transient NRT_EXEC_UNIT_UNRECOVERABLE device wedges clear on retry
        res = run_bass_kernel_spmd(
            nc, in_maps, core_ids=list(range(NCORES)), trace=trace
        )
    LAST_EXEC_TIME_NS = res.exec_time_ns

    out = np.zeros((B, 1, N_NODES, 1), dtype=np.float32)
    hn = np.zeros((N_NODES, 2, B, HID), dtype=np.float32)
    cn = np.zeros((N_NODES, 2, B, HID), dtype=np.float32)
    for c in range(NCORES):
        r = res.results[c]
        nlo, nhi = c * NN, min((c + 1) * NN, N_NODES)
        cnt = nhi - nlo

        # outp: [1, p*128 + s*64 + b] -> [node, b]
        o = r["outp"].reshape(PAIRS, 2, B).reshape(NN, B)
        out[:, 0, nlo:nhi, 0] = o[:cnt].T

        def hstates(a):  # bf16 [128, P*128] -> [node, b, h]
            v = np.asarray(a, dtype=np.float32).reshape(2, B, PAIRS, PB)
            return v[..., 0:HID].transpose(2, 0, 1, 3).reshape(NN, B, HID)

        def cstates(a):  # f32 [128, P*100] -> [node, b, h]
            v = a.reshape(2, B, PAIRS, HID)
            return v.transpose(2, 0, 1, 3).reshape(NN, B, HID)

        hn[nlo:nhi, 0] = hstates(r["h1f"])[:cnt]
        hn[nlo:nhi, 1] = hstates(r["h2f"])[:cnt]
        cn[nlo:nhi, 0] = cstates(r["c1f"])[:cnt]
        cn[nlo:nhi, 1] = cstates(r["c2f"])[:cnt]
    return (out, hn, cn)
